# revision 33
# baseline (speedup 1.0000x reference)
"""AlignmentAttention Trainium2 kernel (8 NeuronCores, pure data parallel over B).

Math: reference computes
    key    = einsum("nbsr,er->nbse", kv, Wk) + bk
    scores = einsum("bte,nbse->nbts", q, key) + mask
    out    = softmax(scores) @ kv
Because softmax is invariant to per-row constants, the bias term q@bk cancels,
and q @ (kv@Wk^T)^T == (q@Wk) @ kv^T.  So we project the *query* once per batch
element (qproj = q@Wk, shared across all N candidates) instead of projecting
keys N times, and drop bk entirely.

Sharding: one batch element b per core (B=8 == n_cores).  Per core:
    qprojT = (q_b @ Wk)^T              64 matmuls   (fp16 operands, f32 psum)
    per candidate n:
        scores  = qproj @ kvT_nb        32 matmuls
        softmax: DVE mask-add + rowmax, ACT fused exp+rowsum -> fp16 attn
        attn^T via fp16 PE transpose (1 cyc/row, fp16 psum)
        out_nb  = attn @ kv_nb          32 matmuls, 1/rowsum fused into the
                  fp16 eviction; host upcasts fp16 -> f32

Perf notes (from perfetto/NTFF analysis):
  - engine preambles delay all real work to ~9us; input DMA starts ~8.7us.
    A few fp16 identity transposes in the dead zone pull the PE p-state
    ramp (0.65 -> 1.2 -> 2.4 GHz) earlier so qproj runs at full clock.
  - chunked input DMAs (not one big load) keep Tile's dependency tracking
    slice-granular, so matmuls start as soon as their chunk lands.
  - kv/kvT pools are 3 deep: candidate n+2's loads overlap candidate n,
    hiding the buffer-reuse serialization that otherwise starves scores.
  - transpose+out matmuls are emitted per-ti so a late softmax(ti=3)
    cannot block ready out-matmuls behind it in the PE queue.
  - out is fp16 on the gpsimd queue: halves tail-store bytes and keeps
    DMA issue off the scalar engine (which runs exp + evictions).
"""
import contextlib
import os
import sys

import numpy as np

_TRN_REPO = "/opt/trn_rl_repo"
if _TRN_REPO not in sys.path and os.path.isdir(_TRN_REPO):
    sys.path.insert(0, _TRN_REPO)

# jax on the native neuron backend crashes; the axon PJRT proxy path needs the
# default platform selection.
if os.environ.get("JAX_PLATFORMS") == "cpu":
    os.environ["JAX_PLATFORMS"] = ""

import concourse.bacc as bacc
import concourse.tile as tile
from concourse import mybir
from concourse.masks import make_identity
from concourse.bass_utils import run_bass_kernel_spmd

F32 = mybir.dt.float32
F16 = mybir.dt.float16

N_CAND, B, T, S, E, R = 4, 8, 512, 512, 1024, 1024
TT, ST, ET, RT = T // 128, S // 128, E // 128, R // 128

_NC_CACHE = []


def build_nc():
    nc = bacc.Bacc(None, target_bir_lowering=False)
    qT = nc.declare_dram_parameter("qT", [E, T], F16, isOutput=False)
    kv = nc.declare_dram_parameter("kv", [N_CAND, S, R], F16, isOutput=False)
    kvT = nc.declare_dram_parameter("kvT", [N_CAND, R, S], F16, isOutput=False)
    mask = nc.declare_dram_parameter("mask", [T, S], F16, isOutput=False)
    wk = nc.declare_dram_parameter("wk", [E, R], F16, isOutput=False)
    out = nc.declare_dram_parameter("out", [N_CAND, T, R], F16, isOutput=True)

    with contextlib.ExitStack() as ctx:
        tc = ctx.enter_context(tile.TileContext(nc))
        singles = ctx.enter_context(tc.tile_pool(name="singles", bufs=1))
        kvpool = ctx.enter_context(tc.tile_pool(name="kvpool", bufs=3))
        kvtpool = ctx.enter_context(tc.tile_pool(name="kvtpool", bufs=3))
        scorepool = ctx.enter_context(tc.tile_pool(name="scorepool", bufs=3))
        attnpool = ctx.enter_context(tc.tile_pool(name="attnpool", bufs=3))
        attntpool = ctx.enter_context(tc.tile_pool(name="attntpool", bufs=2))
        outpool = ctx.enter_context(tc.tile_pool(name="outpool", bufs=3))
        smalls = ctx.enter_context(tc.tile_pool(name="smalls", bufs=10))
        psT = ctx.enter_context(tc.tile_pool(name="psT", bufs=2, space="PSUM"))
        psmm = ctx.enter_context(tc.tile_pool(name="psmm", bufs=6, space="PSUM"))

        ident16 = singles.tile([128, 128], F16)
        make_identity(nc, ident16)

        # Dead-zone warmup: the engine preamble blocks real work until ~9us
        # anyway; these transposes are free (PE would idle) and carry the PE
        # p-state ramp (full clock needs ~3us of *uninterrupted* activity)
        # through to the first data-dependent matmul.  Two tiles cycling 8
        # distinct slices keep WAW deps 8 instructions back.
        wps = [psT.tile([128, 512], F16, tag="pT", name=f"wp{i}") for i in range(2)]
        for k in range(16):
            wp = wps[(k // 4) % 2]
            nc.tensor.transpose(wp[:, (k % 4) * 128:(k % 4 + 1) * 128],
                                ident16, ident16)

        # persistent SBUF.  Each dma_start costs ~600ns of engine issue time,
        # so batch chunks: qT in 2 DMAs, wk in 4 (interleaved so qproj's
        # e-major sweep can chase arrivals pairwise).
        wk_sb = singles.tile([128, ET, R], F16)
        qT_sb = singles.tile([128, ET, T], F16)
        # qT on the sync queue, wk on the scalar queue: two in-order DMA
        # queues stream in parallel, so the e-major qproj sweep is never
        # starved waiting for one serialized queue to reach the wk chunks.
        nc.sync.dma_start(out=qT_sb[:, 0, :], in_=qT[0:128, :])
        nc.scalar.dma_start(out=wk_sb[:, 0, :], in_=wk[0:128, :])
        nc.sync.dma_start(out=qT_sb[:, 1:4, :],
                          in_=qT[128:512, :].rearrange("(eh p) t -> p eh t", p=128))
        nc.scalar.dma_start(out=wk_sb[:, 1:4, :],
                            in_=wk[128:512, :].rearrange("(eh p) r -> p eh r", p=128))
        nc.sync.dma_start(out=qT_sb[:, 4:8, :],
                          in_=qT[512:1024, :].rearrange("(eh p) t -> p eh t", p=128))
        nc.scalar.dma_start(out=wk_sb[:, 4:8, :],
                            in_=wk[512:1024, :].rearrange("(eh p) r -> p eh r", p=128))
        mask_sb = singles.tile([128, TT, S], F16)
        qprojT = singles.tile([128, RT, T], F16)

        # qprojT[r, t] = sum_e wk[e, r] * qT[e, t]
        # e-major in two passes of 6+2 r-blocks (6 live psum banks): the mm
        # stream chases the wk/qT chunk arrivals instead of being gated on
        # the last chunk for every r-block.
        for r0, r1 in ((0, 6), (6, RT)):
            ps = {r: psmm.tile([128, T], F32, name=f"qp{r}", tag="p")
                  for r in range(r0, r1)}
            for e in range(ET):
                for r in range(r0, r1):
                    nc.tensor.matmul(ps[r], wk_sb[:, e, r * 128:(r + 1) * 128],
                                     qT_sb[:, e, :],
                                     start=(e == 0), stop=(e == ET - 1))
            for r in range(r0, r1):
                nc.scalar.copy(qprojT[:, r, :], ps[r])

        # mask is needed only at the first softmax; keep it off the critical
        # prologue path.
        nc.sync.dma_start(out=mask_sb,
                          in_=mask.rearrange("(th p) s -> p th s", p=128))

        for n in range(N_CAND):
            kvT_sb = kvtpool.tile([128, RT, S], F16)
            for h in range(2):
                nc.sync.dma_start(
                    out=kvT_sb[:, 4 * h:4 * h + 4, :],
                    in_=kvT[n, 512 * h:512 * (h + 1), :].rearrange(
                        "(rh p) s -> p rh s", p=128))
            kv_sb = kvpool.tile([128, ST, R], F16)
            for h in range(2):
                nc.sync.dma_start(
                    out=kv_sb[:, 2 * h:2 * h + 2, :],
                    in_=kv[n, 256 * h:256 * (h + 1), :].rearrange(
                        "(sh p) r -> p sh r", p=128))

            # Software-pipelined emission.  PE program order per candidate is
            #   S0 S1 S2 [T0] S3 [T1] O0 [T2] O1 [T3] O2 O3
            # so each transpose block T(ti) runs well before its outs O(ti)
            # need the attnT copy — the scores-end -> first-out critical path
            # has no transpose/copy latency on it.  Engine side streams:
            #   DVE: add/red per ti interleaved with copies and recips
            #   ACT: exp per ti, then the out evictions
            score_ps = [None] * TT
            attns = [None] * TT
            sumexps = [None] * TT
            recips = [None] * TT
            attnT = attntpool.tile([128, ST, T], F16)

            def scores_mms(ti):
                p = psmm.tile([128, S], F32, name="p")
                for ri in range(RT):
                    nc.tensor.matmul(p, qprojT[:, ri, ti * 128:(ti + 1) * 128],
                                     kvT_sb[:, ri, :],
                                     start=(ri == 0), stop=(ri == RT - 1))
                score_ps[ti] = p

            def softmax(ti):
                # unnormalized: attn_u = exp(scores + mask - rowmax) in fp16;
                # 1/rowsum is deferred to the out-matmul eviction
                scoresN = scorepool.tile([128, S], F32, name="scoresN")
                negmax = smalls.tile([128, 1], F32, name="negmax")
                nc.vector.tensor_add(scoresN, score_ps[ti], mask_sb[:, ti, :])
                nc.vector.tensor_reduce(negmax, scoresN, axis=mybir.AxisListType.X,
                                        op=mybir.AluOpType.max, negate=True)
                attn = attnpool.tile([128, S], F16, name="attn")
                sumexp = smalls.tile([128, 1], F32, name="sumexp")
                nc.scalar.activation(attn, scoresN, mybir.ActivationFunctionType.Exp,
                                     bias=negmax, scale=1.0, accum_out=sumexp)
                attns[ti] = attn
                sumexps[ti] = sumexp

            def transpose_copy(ti):
                pT = psT.tile([128, 512], F16, name="pT", tag="pT")
                for si in range(ST):
                    nc.tensor.transpose(pT[:, si * 128:(si + 1) * 128],
                                        attns[ti][:, si * 128:(si + 1) * 128],
                                        ident16)
                nc.vector.tensor_copy(
                    attnT[:, 0:ST, ti * 128:(ti + 1) * 128],
                    pT.rearrange("p (k j) -> p k j", k=ST))
                recip = smalls.tile([128, 1], F32, name="recip")
                nc.vector.reciprocal(recip, sumexps[ti])
                recips[ti] = recip

            def out_mms(ti):
                # out[t, r] = sum_s attn_u[t, s] kv[s, r]; normalize on eviction
                for rh in range(2):
                    p = psmm.tile([128, 512], F32, name="p")
                    for si in range(ST):
                        nc.tensor.matmul(p, attnT[:, si, ti * 128:(ti + 1) * 128],
                                         kv_sb[:, si, rh * 512:(rh + 1) * 512],
                                         start=(si == 0), stop=(si == ST - 1))
                    o = outpool.tile([128, 512], F16, name="o")
                    nc.scalar.mul(o, p, recips[ti])
                    nc.gpsimd.dma_start(
                        out=out[n, ti * 128:(ti + 1) * 128, rh * 512:(rh + 1) * 512],
                        in_=o)

            scores_mms(0)
            softmax(0)
            scores_mms(1)
            softmax(1)
            scores_mms(2)
            softmax(2)
            transpose_copy(0)
            scores_mms(3)
            softmax(3)
            transpose_copy(1)
            out_mms(0)
            transpose_copy(2)
            out_mms(1)
            transpose_copy(3)
            out_mms(2)
            out_mms(3)

    nc.compile()
    return nc


def make_in_maps(query, key_value_states, attention_mask, Wk):
    in_maps = []
    for b in range(B):
        in_maps.append({
            "qT": np.ascontiguousarray(query[0, b].T).astype(np.float16),
            "kv": np.ascontiguousarray(key_value_states[:, b]).astype(np.float16),
            "kvT": np.ascontiguousarray(
                key_value_states[:, b].transpose(0, 2, 1)).astype(np.float16),
            "mask": np.ascontiguousarray(attention_mask[0, b]).astype(np.float16),
            "wk": np.ascontiguousarray(Wk).astype(np.float16),
        })
    return in_maps


def kernel(query, key_value_states, attention_mask, Wk, bk):
    query = np.asarray(query, dtype=np.float32)
    key_value_states = np.asarray(key_value_states, dtype=np.float32)
    attention_mask = np.asarray(attention_mask, dtype=np.float32)
    Wk = np.asarray(Wk, dtype=np.float32)
    del bk  # cancels inside the softmax (constant along the softmax axis)

    if not _NC_CACHE:
        _NC_CACHE.append(build_nc())
    nc = _NC_CACHE[0]

    in_maps = make_in_maps(query, key_value_states, attention_mask, Wk)
    res = run_bass_kernel_spmd(nc, in_maps, core_ids=list(range(B)))

    out = np.empty((N_CAND, B, T, R), dtype=np.float32)
    for b in range(B):
        out[:, b] = res.results[b]["out"].astype(np.float32)
    return out
